# revision 2
# baseline (speedup 1.0000x reference)
"""CenterLoss kernel for Trainium2 (Bass/Tile), 8-core data-parallel.

loss = sum_i ||x_i - centers[labels_i]||^2
  x: (65536, 512) f32, labels: (65536,) int, centers: (512, 512) f32

Per-core plan (8192 rows each), using the expansion
  loss = sum x^2 - 2*sum_{c,d} S[c,d]*centers[c,d] + sum_c count_c*||C_c||^2
with S = onehot(labels)^T @ x computed on the PE via one-hot matmuls and
count_c precomputed on host (np.bincount of the int labels).

Pipeline per core:
  - x streamed HBM->SBUF as plain f32 via HWDGE (sync queue) in 6 chunks
    (1/3/4/4/3/1 MB), contiguous-per-partition layout for max DMA rate
  - GpSimd casts each 512-row slab f32 -> fp8e4m3 for the PE
  - DVE builds one-hot tiles: is_equal(iota_row, label_p)
  - PE: per 256-row group, 4 DoubleRow matmuls accumulate S into PSUM
  - ACT accumulates sum(x^2) per 512-row slab from the f32 data
  - tail: r2 = -2*sum(S.*C) on DVE, r3 = sum_c count_c*||C_c||^2 from the
    host-provided counts and on-chip csq, r1 = sum x^2; out = r1+r2+r3
    as [128,1] per-core partials; host sums.
"""

import sys

import numpy as np

sys.path.insert(0, "/opt/trn_rl_repo")

N_CORES = 8
B = 65536
D = 512
B_L = B // N_CORES  # 8192 rows per core

# x chunk sizes in rows (DMA granularity); small edges for pipeline ramp
CHUNK_ROWS = [512, 1536, 2048, 2048, 1536, 512]
assert sum(CHUNK_ROWS) == B_L
NCH = D // 128  # 4 class chunks
SLAB = 512  # rows per cast / sumsq op
N_SLABS = B_L // SLAB  # 16

_CACHE = {}


def _build():
    """Trace the Bass/Tile program once; returns the compiled Bacc module."""
    if "nc" in _CACHE:
        return _CACHE["nc"]

    import concourse.bacc as bacc
    import concourse.mybir as mybir
    import concourse.tile as tile

    f32 = mybir.dt.float32
    fp8 = mybir.dt.float8e4

    nc = bacc.Bacc("TRN2", debug=False, num_devices=N_CORES)
    x_t = nc.dram_tensor("x", [B_L, D], f32, kind="ExternalInput")
    iota_t = nc.dram_tensor("iota16", [128, D], mybir.dt.float16, kind="ExternalInput")
    labf_t = nc.dram_tensor("labf", [128, B_L // 128], f32, kind="ExternalInput")
    cnt_t = nc.dram_tensor("counts", [128, NCH], f32, kind="ExternalInput")
    c_t = nc.dram_tensor("centers", [D, D], f32, kind="ExternalInput")
    out_t = nc.dram_tensor("out", [128, 1], f32, kind="ExternalOutput")

    with tile.TileContext(nc) as tc:
        with (
            tc.tile_pool(name="misc", bufs=1) as misc_pool,
            tc.tile_pool(name="psum", bufs=1, space="PSUM") as psum_pool,
        ):
            # small inputs on the scalar HWDGE ring (parallel to x stream)
            iota_sb = misc_pool.tile([128, D], mybir.dt.float16)
            nc.scalar.dma_start(iota_sb[:], iota_t.ap())
            labf_sb = misc_pool.tile([128, B_L // 128], f32)
            nc.scalar.dma_start(labf_sb[:], labf_t.ap())
            cnt_sb = misc_pool.tile([128, NCH], f32)
            nc.scalar.dma_start(cnt_sb[:], cnt_t.ap())
            cent_sb = misc_pool.tile([128, NCH, D], f32)
            nc.scalar.dma_start(
                cent_sb[:], c_t.ap().rearrange("(n p) d -> p n d", p=128)
            )

            acc_x2 = misc_pool.tile([128, N_SLABS], f32)
            csq_col = misc_pool.tile([128, NCH], f32)
            junk_dve = misc_pool.tile([128, 1], f32)
            junk_act = misc_pool.tile([128, 1], f32)
            r1 = misc_pool.tile([128, 1], f32)
            r2 = misc_pool.tile([128, 1], f32)
            r3 = misc_pool.tile([128, 1], f32)

            # csq[c-chunk] = rowsum(centers^2), consumed by the tail only
            for c in range(NCH):
                nc.scalar.activation(
                    junk_act[:].broadcast_to(cent_sb[:, c, :].shape),
                    cent_sb[:, c, :],
                    mybir.ActivationFunctionType.Square,
                    accum_out=csq_col[:, c : c + 1],
                )
            # r3 = sum_c count_c * csq_c (host-precomputed histogram)
            nc.vector.scalar_tensor_tensor(
                out=junk_dve[:].broadcast_to(cnt_sb[:].shape),
                in0=cnt_sb[:],
                scalar=1.0,
                in1=csq_col[:],
                op0=mybir.AluOpType.bypass,
                op1=mybir.AluOpType.mult,
                accum_out=r3[:],
            )

            S_all = psum_pool.tile([128, NCH, D], f32, name="S_all")
            S_ps = [S_all[:, c, :] for c in range(NCH)]

            # static x tiles per chunk: f32 landing pad + fp8 cast output
            x32 = [
                misc_pool.tile([128, r // 128, D], f32, name=f"x32_{i}")
                for i, r in enumerate(CHUNK_ROWS)
            ]
            x8 = [
                misc_pool.tile([128, r // 128, D], fp8, name=f"x8_{i}")
                for i, r in enumerate(CHUNK_ROWS)
            ]

            x_ap = x_t.ap()
            n_groups = B_L // 256  # 32 DoubleRow matmul groups
            g = 0  # global group index
            t = 0  # global q-column index (labf column)
            slab = 0  # global 512-row slab index
            lo = 0
            for ci, rows in enumerate(CHUNK_ROWS):
                qc = rows // 128
                # contiguous-per-partition layout: partition p holds rows
                # [lo + p*qc, lo + (p+1)*qc)
                nc.sync.dma_start(
                    x32[ci][:],
                    x_ap[lo : lo + rows, :].rearrange("(p q) d -> p q d", p=128),
                )
                # per 512-row slab: fp8 cast (GpSimd) + sum(x^2) (ACT, f32)
                for k in range(qc // 4):
                    sl = slice(4 * k, 4 * k + 4)
                    nc.gpsimd.tensor_copy(x8[ci][:, sl, :], x32[ci][:, sl, :])
                    x_flat = x32[ci][:, sl, :].rearrange("p q d -> p (q d)")
                    nc.scalar.activation(
                        junk_act[:].broadcast_to(x_flat.shape),
                        x_flat,
                        mybir.ActivationFunctionType.Square,
                        accum_out=acc_x2[:, slab : slab + 1],
                    )
                    slab += 1
                # per 256-row group: one-hot build + 4 DoubleRow matmuls
                for j in range(qc // 2):
                    oh = misc_pool.tile([128, 2, D], fp8, tag="oh", bufs=12)
                    for u in range(2):
                        nc.vector.tensor_scalar(
                            out=oh[:, u, :],
                            in0=iota_sb[:],
                            scalar1=labf_sb[:, t : t + 1],
                            scalar2=None,
                            op0=mybir.AluOpType.is_equal,
                        )
                        t += 1
                    first = g == 0
                    last = g == n_groups - 1
                    for c in range(NCH):
                        nc.tensor.matmul(
                            S_ps[c],
                            lhsT=oh[:, :, c * 128 : (c + 1) * 128],
                            rhs=x8[ci][:, 2 * j : 2 * j + 2, :],
                            start=first,
                            stop=last,
                            perf_mode=mybir.MatmulPerfMode.DoubleRow,
                        )
                    g += 1
                lo += rows

            # tail: r2 = -2*sum_{c,d} S[c,d]*C[c,d] in one fused DVE op
            S_flat = S_all[:].rearrange("p c d -> p (c d)")
            C_flat = cent_sb[:].rearrange("p c d -> p (c d)")
            nc.vector.scalar_tensor_tensor(
                out=junk_dve[:].broadcast_to(S_flat.shape),
                in0=S_flat,
                scalar=-2.0,
                in1=C_flat,
                op0=mybir.AluOpType.mult,
                op1=mybir.AluOpType.mult,
                accum_out=r2[:],
            )
            nc.vector.tensor_reduce(
                r1[:], acc_x2[:], axis=mybir.AxisListType.X, op=mybir.AluOpType.add
            )
            nc.vector.tensor_tensor(r1[:], r1[:], r2[:], op=mybir.AluOpType.add)
            nc.vector.tensor_tensor(r1[:], r1[:], r3[:], op=mybir.AluOpType.add)
            nc.sync.dma_start(out_t.ap(), r1[:])

    nc.compile()
    _CACHE["nc"] = nc
    return nc


def _prep_inputs(x, labels, centers):
    """Shard full inputs into the 8 per-core input maps."""
    x = np.asarray(x, dtype=np.float32)
    labels = np.asarray(labels)
    centers = np.ascontiguousarray(np.asarray(centers, dtype=np.float32))
    iota16 = np.ascontiguousarray(np.tile(np.arange(D, dtype=np.float16), (128, 1)))
    in_maps = []
    for cidx in range(N_CORES):
        xs = np.ascontiguousarray(x[cidx * B_L : (cidx + 1) * B_L])
        lab = np.asarray(labels[cidx * B_L : (cidx + 1) * B_L], dtype=np.int64)
        # labf[p, t]: label of the row that lands at (partition p, q-col t),
        # chunk ci contributing qc = rows/128 q-cols, row = lo + p*qc + qq
        cols = []
        lo = 0
        for rows in CHUNK_ROWS:
            qc = rows // 128
            cols.append(lab[lo : lo + rows].reshape(128, qc))
            lo += rows
        labf = np.ascontiguousarray(np.concatenate(cols, axis=1).astype(np.float32))
        # counts[p, ch] = #{labels == ch*128 + p} (histogram of int indices)
        bc = np.bincount(lab, minlength=D).astype(np.float32)
        counts = np.ascontiguousarray(bc.reshape(NCH, 128).T)
        in_maps.append(
            {
                "x": xs,
                "iota16": iota16,
                "labf": labf,
                "counts": counts,
                "centers": centers,
            }
        )
    return in_maps


def _run(x, labels, centers, trace=False):
    from concourse import bass_utils

    nc = _build()
    in_maps = _prep_inputs(x, labels, centers)
    res = bass_utils.run_bass_kernel_spmd(
        nc, in_maps, core_ids=list(range(N_CORES)), trace=trace
    )
    total = np.float64(0.0)
    for r in res.results:
        total += np.sum(r["out"].astype(np.float64))
    return np.array(total, dtype=np.float32), res


def kernel(x, labels, centers):
    out, _ = _run(x, labels, centers, trace=False)
    return out


def kernel_traced(x, labels, centers):
    return _run(x, labels, centers, trace=True)


# revision 5
# speedup vs baseline: 1.4729x; 1.4729x over previous
"""CenterLoss kernel for Trainium2 (Bass/Tile), 8-core data-parallel.

loss = sum_i ||x_i - centers[labels_i]||^2
  x: (65536, 512) f32, labels: (65536,) int, centers: (512, 512) f32

Per-core plan (8192 rows each), using the expansion
  loss = sum x^2 - 2*sum_{c,d} S[c,d]*centers[c,d] + sum_c count_c*||C_c||^2
with S = onehot(labels)^T @ x computed on the PE via one-hot matmuls and
count_c precomputed on host (np.bincount of the int labels).

Pipeline per core:
  - x streamed HBM->SBUF as plain f32 via HWDGE (sync queue) in chunks,
    contiguous-per-partition layout for max DMA rate (~415 GB/s)
  - f32 -> fp8e4m3 cast for the PE, slabs split between ACT (activation
    Copy) and DVE (tensor_copy); GpSimd is NOT used (Pool compute is 4x
    slow and locks DVE out of its fast SBUF modes)
  - DVE builds one-hot tiles: is_equal(iota_row, label_p)
  - PE: per 256-row group, 4 DoubleRow matmuls accumulate S into PSUM
  - sum(x^2) from f32 slabs: DVE scalar_tensor_tensor / ACT Square,
    split to balance engine load
  - tail: r2 = -2*sum(S.*C) on DVE, r3 = sum_c count_c*||C_c||^2 from
    host-provided counts and on-chip csq; out = r1+r2+r3 as [128,1]
    per-core partials; host sums.
"""

import sys

import numpy as np

sys.path.insert(0, "/opt/trn_rl_repo")

N_CORES = 8
B = 65536
D = 512
B_L = B // N_CORES  # 8192 rows per core

# x chunk sizes in rows (DMA granularity); small edges for pipeline ramp
CHUNK_ROWS = [512, 1536, 2048, 2048, 1536, 256, 256]
assert sum(CHUNK_ROWS) == B_L
NCH = D // 128  # 4 class chunks

_CACHE = {}


def _build():
    """Trace the Bass/Tile program once; returns the compiled Bacc module."""
    if "nc" in _CACHE:
        return _CACHE["nc"]

    import concourse.bacc as bacc
    import concourse.mybir as mybir
    import concourse.tile as tile

    f32 = mybir.dt.float32
    fp8 = mybir.dt.float8e4

    nc = bacc.Bacc("TRN2", debug=False, num_devices=N_CORES)
    x_t = nc.dram_tensor("x", [B_L, D], f32, kind="ExternalInput")
    iota_t = nc.dram_tensor("iota16", [128, D], mybir.dt.float16, kind="ExternalInput")
    labf_t = nc.dram_tensor("labf", [128, B_L // 128], f32, kind="ExternalInput")
    cnt_t = nc.dram_tensor("counts", [128, NCH], f32, kind="ExternalInput")
    c_t = nc.dram_tensor("centers", [D, D], f32, kind="ExternalInput")
    out_t = nc.dram_tensor("out", [128, 1], f32, kind="ExternalOutput")

    with tile.TileContext(nc) as tc:
        with (
            tc.tile_pool(name="misc", bufs=1) as misc_pool,
            tc.tile_pool(name="psum", bufs=1, space="PSUM") as psum_pool,
        ):
            # small inputs on the scalar HWDGE ring (parallel to x stream)
            iota_sb = misc_pool.tile([128, D], mybir.dt.float16)
            nc.scalar.dma_start(iota_sb[:], iota_t.ap())
            labf_sb = misc_pool.tile([128, B_L // 128], f32)
            nc.scalar.dma_start(labf_sb[:], labf_t.ap())
            cnt_sb = misc_pool.tile([128, NCH], f32)
            nc.scalar.dma_start(cnt_sb[:], cnt_t.ap())
            cent_sb = misc_pool.tile([128, NCH, D], f32)
            nc.scalar.dma_start(
                cent_sb[:], c_t.ap().rearrange("(n p) d -> p n d", p=128)
            )

            acc_x2 = misc_pool.tile([128, 20], f32)
            csq_col = misc_pool.tile([128, NCH], f32)
            junk_dve = misc_pool.tile([128, 1], f32)
            junk_act = misc_pool.tile([128, 1], f32)
            r1 = misc_pool.tile([128, 1], f32)
            r2 = misc_pool.tile([128, 1], f32)
            r3 = misc_pool.tile([128, 1], f32)

            # csq[c-chunk] = rowsum(centers^2), consumed by the tail only
            for c in range(NCH):
                nc.scalar.activation(
                    junk_act[:].broadcast_to(cent_sb[:, c, :].shape),
                    cent_sb[:, c, :],
                    mybir.ActivationFunctionType.Square,
                    accum_out=csq_col[:, c : c + 1],
                )
            # r3 = sum_c count_c * csq_c (host-precomputed histogram)
            nc.vector.scalar_tensor_tensor(
                out=junk_dve[:].broadcast_to(cnt_sb[:].shape),
                in0=cnt_sb[:],
                scalar=1.0,
                in1=csq_col[:],
                op0=mybir.AluOpType.bypass,
                op1=mybir.AluOpType.mult,
                accum_out=r3[:],
            )

            S_all = psum_pool.tile([128, NCH, D], f32, name="S_all")
            S_ps = [S_all[:, c, :] for c in range(NCH)]

            # static x tiles per chunk: f32 landing pad + fp8 cast output
            x32 = [
                misc_pool.tile([128, r // 128, D], f32, name=f"x32_{i}")
                for i, r in enumerate(CHUNK_ROWS)
            ]
            x8 = [
                misc_pool.tile([128, r // 128, D], fp8, name=f"x8_{i}")
                for i, r in enumerate(CHUNK_ROWS)
            ]

            def cast_slab(ci, sl, on_act):
                if on_act:
                    nc.scalar.activation(
                        x8[ci][:, sl, :],
                        x32[ci][:, sl, :],
                        mybir.ActivationFunctionType.Copy,
                    )
                else:
                    nc.vector.tensor_copy(x8[ci][:, sl, :], x32[ci][:, sl, :])

            def sumsq_slab(ci, sl, acc_idx, on_act):
                xf = x32[ci][:, sl, :].rearrange("p q d -> p (q d)")
                if on_act:
                    nc.scalar.activation(
                        junk_act[:].broadcast_to(xf.shape),
                        xf,
                        mybir.ActivationFunctionType.Square,
                        accum_out=acc_x2[:, acc_idx : acc_idx + 1],
                    )
                else:
                    nc.vector.scalar_tensor_tensor(
                        out=junk_dve[:].broadcast_to(xf.shape),
                        in0=xf,
                        scalar=1.0,
                        in1=xf,
                        op0=mybir.AluOpType.bypass,
                        op1=mybir.AluOpType.mult,
                        accum_out=acc_x2[:, acc_idx : acc_idx + 1],
                    )

            x_ap = x_t.ap()
            n_groups = B_L // 256  # 32 DoubleRow matmul groups
            g = 0  # global group index
            t = 0  # global q-column index (labf column)
            slab0 = 0  # global slab base index per chunk
            lo = 0
            for ci, rows in enumerate(CHUNK_ROWS):
                qc = rows // 128
                # contiguous-per-partition layout: partition p holds rows
                # [lo + p*qc, lo + (p+1)*qc)
                nc.sync.dma_start(
                    x32[ci][:],
                    x_ap[lo : lo + rows, :].rearrange("(p q) d -> p q d", p=128),
                )
                # fp8 cast per 512-row slab (or the whole chunk if smaller),
                # alternating ACT/DVE; the final small chunks go to ACT so
                # DVE is free for the tail contraction
                n_sl = max(1, qc // 4)
                for k in range(n_sl):
                    sl = slice(4 * k, min(4 * k + 4, qc))
                    cast_slab(ci, sl, on_act=((slab0 + k) % 2 == 0) or ci >= 5)
                # per 256-row group: one-hot build + 4 DoubleRow matmuls
                for j in range(qc // 2):
                    oh = misc_pool.tile([128, 2, D], fp8, tag="oh", bufs=12)
                    for u in range(2):
                        nc.vector.tensor_scalar(
                            out=oh[:, u, :],
                            in0=iota_sb[:],
                            scalar1=labf_sb[:, t : t + 1],
                            scalar2=None,
                            op0=mybir.AluOpType.is_equal,
                        )
                        t += 1
                    first = g == 0
                    last = g == n_groups - 1
                    for c in range(NCH):
                        nc.tensor.matmul(
                            S_ps[c],
                            lhsT=oh[:, :, c * 128 : (c + 1) * 128],
                            rhs=x8[ci][:, 2 * j : 2 * j + 2, :],
                            start=first,
                            stop=last,
                            perf_mode=mybir.MatmulPerfMode.DoubleRow,
                        )
                    g += 1
                # sum(x^2) per slab from f32, opposite engine of the cast
                for k in range(n_sl):
                    sl = slice(4 * k, min(4 * k + 4, qc))
                    sumsq_slab(
                        ci, sl, slab0 + k, on_act=((slab0 + k) % 2 == 1) and ci < 5
                    )
                slab0 += n_sl
                lo += rows

            # tail: r2 = -2*sum_{c,d} S[c,d]*C[c,d] in one fused DVE op
            S_flat = S_all[:].rearrange("p c d -> p (c d)")
            C_flat = cent_sb[:].rearrange("p c d -> p (c d)")
            nc.vector.scalar_tensor_tensor(
                out=junk_dve[:].broadcast_to(S_flat.shape),
                in0=S_flat,
                scalar=-2.0,
                in1=C_flat,
                op0=mybir.AluOpType.mult,
                op1=mybir.AluOpType.mult,
                accum_out=r2[:],
            )
            nc.vector.tensor_reduce(
                r1[:], acc_x2[:], axis=mybir.AxisListType.X, op=mybir.AluOpType.add
            )
            nc.vector.tensor_tensor(r1[:], r1[:], r2[:], op=mybir.AluOpType.add)
            nc.vector.tensor_tensor(r1[:], r1[:], r3[:], op=mybir.AluOpType.add)
            nc.sync.dma_start(out_t.ap(), r1[:])

    nc.compile()
    _CACHE["nc"] = nc
    return nc


def _prep_inputs(x, labels, centers):
    """Shard full inputs into the 8 per-core input maps."""
    x = np.asarray(x, dtype=np.float32)
    labels = np.asarray(labels)
    centers = np.ascontiguousarray(np.asarray(centers, dtype=np.float32))
    iota16 = np.ascontiguousarray(np.tile(np.arange(D, dtype=np.float16), (128, 1)))
    in_maps = []
    for cidx in range(N_CORES):
        xs = np.ascontiguousarray(x[cidx * B_L : (cidx + 1) * B_L])
        lab = np.asarray(labels[cidx * B_L : (cidx + 1) * B_L], dtype=np.int64)
        # labf[p, t]: label of the row that lands at (partition p, q-col t),
        # chunk ci contributing qc = rows/128 q-cols, row = lo + p*qc + qq
        cols = []
        lo = 0
        for rows in CHUNK_ROWS:
            qc = rows // 128
            cols.append(lab[lo : lo + rows].reshape(128, qc))
            lo += rows
        labf = np.ascontiguousarray(np.concatenate(cols, axis=1).astype(np.float32))
        # counts[p, ch] = #{labels == ch*128 + p} (histogram of int indices)
        bc = np.bincount(lab, minlength=D).astype(np.float32)
        counts = np.ascontiguousarray(bc.reshape(NCH, 128).T)
        in_maps.append(
            {
                "x": xs,
                "iota16": iota16,
                "labf": labf,
                "counts": counts,
                "centers": centers,
            }
        )
    return in_maps


def _run(x, labels, centers, trace=False):
    from concourse import bass_utils

    nc = _build()
    in_maps = _prep_inputs(x, labels, centers)
    res = bass_utils.run_bass_kernel_spmd(
        nc, in_maps, core_ids=list(range(N_CORES)), trace=trace
    )
    total = np.float64(0.0)
    for r in res.results:
        total += np.sum(r["out"].astype(np.float64))
    return np.array(total, dtype=np.float32), res


def kernel(x, labels, centers):
    out, _ = _run(x, labels, centers, trace=False)
    return out


def kernel_traced(x, labels, centers):
    return _run(x, labels, centers, trace=True)


# revision 6
# speedup vs baseline: 1.9081x; 1.2955x over previous
"""CenterLoss kernel for Trainium2 (Bass/Tile), 8-core data-parallel.

loss = sum_i ||x_i - centers[labels_i]||^2
  x: (65536, 512) f32, labels: (65536,) int, centers: (512, 512) f32

Per-core plan (8192 rows each), using the expansion
  loss = sum x^2 - 2*sum_{c,d} S[c,d]*centers[c,d] + sum_c count_c*||C_c||^2
with S = onehot(labels)^T @ x computed on the PE via one-hot matmuls and
count_c precomputed on host (np.bincount of the int labels).

Pipeline per core:
  - x streamed HBM->SBUF as plain f32 via HWDGE (sync queue) in 6 chunks,
    contiguous-per-partition layout for max DMA rate (~415 GB/s); six
    sync-ring DMAs + the output keep within the 8 HWDGE semaphore lanes
  - small inputs (iota/labf/counts/centers) via the gpsimd SWDGE queue so
    they never share a completion lane with the bulk x stream
  - f32 -> fp8e4m3 cast for the PE: DVE tensor_copy (2x mode) for most
    slabs, a few on ACT to balance load; GpSimd compute is NOT used
    (Pool ops are 4x slow and lock DVE out of its fast SBUF modes)
  - DVE builds one-hot tiles: is_equal(iota_row, label_p)
  - PE: per 256-row group, 4 DoubleRow matmuls accumulate S into PSUM
  - ACT accumulates sum(x^2) per chunk from the f32 data
  - tail: r2 = -2*sum(S.*C) on DVE, r3 = sum_c count_c*||C_c||^2 from
    host-provided counts and on-chip csq; out = r1+r2+r3 as [128,1]
    per-core partials; host sums.
"""

import sys

import numpy as np

sys.path.insert(0, "/opt/trn_rl_repo")

N_CORES = 8
B = 65536
D = 512
B_L = B // N_CORES  # 8192 rows per core

# x chunk sizes in rows (DMA granularity); small edges for pipeline ramp
CHUNK_ROWS = [512, 1536, 2048, 2048, 1792, 256]
assert sum(CHUNK_ROWS) == B_L
NCH = D // 128  # 4 class chunks

_CACHE = {}


def _build():
    """Trace the Bass/Tile program once; returns the compiled Bacc module."""
    if "nc" in _CACHE:
        return _CACHE["nc"]

    import concourse.bacc as bacc
    import concourse.mybir as mybir
    import concourse.tile as tile

    f32 = mybir.dt.float32
    fp8 = mybir.dt.float8e4

    nc = bacc.Bacc("TRN2", debug=False, num_devices=N_CORES)
    x_t = nc.dram_tensor("x", [B_L, D], f32, kind="ExternalInput")
    iota_t = nc.dram_tensor("iota16", [128, D], mybir.dt.float16, kind="ExternalInput")
    labf_t = nc.dram_tensor("labf", [128, B_L // 128], f32, kind="ExternalInput")
    cnt_t = nc.dram_tensor("counts", [128, NCH], f32, kind="ExternalInput")
    c_t = nc.dram_tensor("centers", [D, D], f32, kind="ExternalInput")
    out_t = nc.dram_tensor("out", [128, 1], f32, kind="ExternalOutput")

    with tile.TileContext(nc) as tc:
        with (
            tc.tile_pool(name="misc", bufs=1) as misc_pool,
            tc.tile_pool(name="psum", bufs=1, space="PSUM") as psum_pool,
        ):
            # small inputs on the SWDGE (gpsimd) queue: separate semaphores
            # from the bulk HWDGE x stream, and the Pool engine is idle
            iota_sb = misc_pool.tile([128, D], mybir.dt.float16)
            nc.gpsimd.dma_start(iota_sb[:], iota_t.ap())
            labf_sb = misc_pool.tile([128, B_L // 128], f32)
            nc.gpsimd.dma_start(labf_sb[:], labf_t.ap())
            cnt_sb = misc_pool.tile([128, NCH], f32)
            nc.gpsimd.dma_start(cnt_sb[:], cnt_t.ap())
            cent_sb = misc_pool.tile([128, NCH, D], f32)
            nc.gpsimd.dma_start(
                cent_sb[:], c_t.ap().rearrange("(n p) d -> p n d", p=128)
            )

            acc_x2 = misc_pool.tile([128, len(CHUNK_ROWS)], f32)
            csq_col = misc_pool.tile([128, NCH], f32)
            junk_dve = misc_pool.tile([128, 1], f32)
            junk_act = misc_pool.tile([128, 1], f32)
            r1 = misc_pool.tile([128, 1], f32)
            r2 = misc_pool.tile([128, 1], f32)
            r3 = misc_pool.tile([128, 1], f32)

            S_all = psum_pool.tile([128, NCH, D], f32, name="S_all")
            S_ps = [S_all[:, c, :] for c in range(NCH)]

            # static x tiles per chunk: f32 landing pad + fp8 cast output
            x32 = [
                misc_pool.tile([128, r // 128, D], f32, name=f"x32_{i}")
                for i, r in enumerate(CHUNK_ROWS)
            ]
            x8 = [
                misc_pool.tile([128, r // 128, D], fp8, name=f"x8_{i}")
                for i, r in enumerate(CHUNK_ROWS)
            ]

            def cast_slab(ci, sl, on_act):
                if on_act:
                    nc.scalar.activation(
                        x8[ci][:, sl, :],
                        x32[ci][:, sl, :],
                        mybir.ActivationFunctionType.Copy,
                    )
                else:
                    nc.vector.tensor_copy(x8[ci][:, sl, :], x32[ci][:, sl, :])

            x_ap = x_t.ap()
            n_groups = B_L // 256  # 32 DoubleRow matmul groups
            g = 0  # global group index
            t = 0  # global q-column index (labf column)
            slab = 0  # global 512-row slab counter (for cast engine choice)
            lo = 0
            for ci, rows in enumerate(CHUNK_ROWS):
                qc = rows // 128
                # contiguous-per-partition layout: partition p holds rows
                # [lo + p*qc, lo + (p+1)*qc)
                nc.sync.dma_start(
                    x32[ci][:],
                    x_ap[lo : lo + rows, :].rearrange("(p q) d -> p q d", p=128),
                )
                # fp8 cast per 512-row slab (or whole small chunk): DVE 2x
                # mode is the cheapest; every 4th slab goes to ACT to
                # balance, the final small chunk back on DVE for the tail
                n_sl = max(1, qc // 4)
                for k in range(n_sl):
                    sl = slice(4 * k, min(4 * k + 4, qc))
                    on_act = (slab % 4 == 3) and ci < len(CHUNK_ROWS) - 1
                    cast_slab(ci, sl, on_act=on_act)
                    slab += 1
                # per 256-row group: one-hot build + 4 DoubleRow matmuls
                for j in range(qc // 2):
                    oh = misc_pool.tile([128, 2, D], fp8, tag="oh", bufs=12)
                    for u in range(2):
                        nc.vector.tensor_scalar(
                            out=oh[:, u, :],
                            in0=iota_sb[:],
                            scalar1=labf_sb[:, t : t + 1],
                            scalar2=None,
                            op0=mybir.AluOpType.is_equal,
                        )
                        t += 1
                    first = g == 0
                    last = g == n_groups - 1
                    for c in range(NCH):
                        nc.tensor.matmul(
                            S_ps[c],
                            lhsT=oh[:, :, c * 128 : (c + 1) * 128],
                            rhs=x8[ci][:, 2 * j : 2 * j + 2, :],
                            start=first,
                            stop=last,
                            perf_mode=mybir.MatmulPerfMode.DoubleRow,
                        )
                    g += 1
                # sum(x^2) for the whole chunk on ACT from the f32 data
                x_flat = x32[ci][:].rearrange("p q d -> p (q d)")
                nc.scalar.activation(
                    junk_act[:].broadcast_to(x_flat.shape),
                    x_flat,
                    mybir.ActivationFunctionType.Square,
                    accum_out=acc_x2[:, ci : ci + 1],
                )
                lo += rows

            # csq[c-chunk] = rowsum(centers^2), consumed by the tail only
            for c in range(NCH):
                nc.scalar.activation(
                    junk_act[:].broadcast_to(cent_sb[:, c, :].shape),
                    cent_sb[:, c, :],
                    mybir.ActivationFunctionType.Square,
                    accum_out=csq_col[:, c : c + 1],
                )
            # r3 = sum_c count_c * csq_c (host-precomputed histogram)
            nc.vector.scalar_tensor_tensor(
                out=junk_dve[:].broadcast_to(cnt_sb[:].shape),
                in0=cnt_sb[:],
                scalar=1.0,
                in1=csq_col[:],
                op0=mybir.AluOpType.bypass,
                op1=mybir.AluOpType.mult,
                accum_out=r3[:],
            )

            # tail: r2 = -2*sum_{c,d} S[c,d]*C[c,d] in one fused DVE op
            S_flat = S_all[:].rearrange("p c d -> p (c d)")
            C_flat = cent_sb[:].rearrange("p c d -> p (c d)")
            nc.vector.scalar_tensor_tensor(
                out=junk_dve[:].broadcast_to(S_flat.shape),
                in0=S_flat,
                scalar=-2.0,
                in1=C_flat,
                op0=mybir.AluOpType.mult,
                op1=mybir.AluOpType.mult,
                accum_out=r2[:],
            )
            nc.vector.tensor_reduce(
                r1[:], acc_x2[:], axis=mybir.AxisListType.X, op=mybir.AluOpType.add
            )
            nc.vector.tensor_tensor(r1[:], r1[:], r2[:], op=mybir.AluOpType.add)
            nc.vector.tensor_tensor(r1[:], r1[:], r3[:], op=mybir.AluOpType.add)
            nc.sync.dma_start(out_t.ap(), r1[:])

    nc.compile()
    _CACHE["nc"] = nc
    return nc


def _prep_inputs(x, labels, centers):
    """Shard full inputs into the 8 per-core input maps."""
    x = np.asarray(x, dtype=np.float32)
    labels = np.asarray(labels)
    centers = np.ascontiguousarray(np.asarray(centers, dtype=np.float32))
    iota16 = np.ascontiguousarray(np.tile(np.arange(D, dtype=np.float16), (128, 1)))
    in_maps = []
    for cidx in range(N_CORES):
        xs = np.ascontiguousarray(x[cidx * B_L : (cidx + 1) * B_L])
        lab = np.asarray(labels[cidx * B_L : (cidx + 1) * B_L], dtype=np.int64)
        # labf[p, t]: label of the row that lands at (partition p, q-col t),
        # chunk ci contributing qc = rows/128 q-cols, row = lo + p*qc + qq
        cols = []
        lo = 0
        for rows in CHUNK_ROWS:
            qc = rows // 128
            cols.append(lab[lo : lo + rows].reshape(128, qc))
            lo += rows
        labf = np.ascontiguousarray(np.concatenate(cols, axis=1).astype(np.float32))
        # counts[p, ch] = #{labels == ch*128 + p} (histogram of int indices)
        bc = np.bincount(lab, minlength=D).astype(np.float32)
        counts = np.ascontiguousarray(bc.reshape(NCH, 128).T)
        in_maps.append(
            {
                "x": xs,
                "iota16": iota16,
                "labf": labf,
                "counts": counts,
                "centers": centers,
            }
        )
    return in_maps


def _run(x, labels, centers, trace=False):
    from concourse import bass_utils

    nc = _build()
    in_maps = _prep_inputs(x, labels, centers)
    res = bass_utils.run_bass_kernel_spmd(
        nc, in_maps, core_ids=list(range(N_CORES)), trace=trace
    )
    total = np.float64(0.0)
    for r in res.results:
        total += np.sum(r["out"].astype(np.float64))
    return np.array(total, dtype=np.float32), res


def kernel(x, labels, centers):
    out, _ = _run(x, labels, centers, trace=False)
    return out


def kernel_traced(x, labels, centers):
    return _run(x, labels, centers, trace=True)
